# revision 35
# baseline (speedup 1.0000x reference)
"""Multi-head causal self-attention (B=2, T=2048, D=1024, H=16, dk=64) on 8
Trainium2 NeuronCores.

Sharding: batch x head-block. Core c handles batch c//4 and heads
[4*(c%4), 4*(c%4)+4). Each core computes its 4 heads' attention plus the
matching column-block of the output projection; the host sums the 4 partial
outputs per batch and adds the output bias.

Per-core device kernel (matmuls run in float32r mode = full PE rate; all
matmul-fed tensors are materialized as float32r):
  - x^T (host pre-transposed) streams in 512-column chunks
  - Q^T = (Wq x^T + bq), K^T likewise   -> [dk=256, T] layout (dk on partitions)
  - V   = x Wv^T + bv                   -> [T, dk] natural layout, with a
    ones-column appended per head tile (gives softmax denominators for free)
  - per head h, per 512-wide q block (interleaved with the next x^T chunk's
    projections so the PE stays busy during ACT-bound attention):
      S^T[kv,q] = K_h^T.T @ Q_h^T  (PE, scores transposed: kv on partitions;
                                    head pairs run on disjoint PE row groups)
      P^T = exp(S^T/8)             (ACT, no max-subtraction needed: |s|<~8;
                                    two kv tiles per op via [128,2,512] PSUM)
      causal zeroing of kv>q       (GpSimd affine_select, diagonal tiles only)
      out^T[dk+1,q] += [V_h|1].T @ P^T   (PE, PSUM-accumulated over kv tiles;
                                          row dk holds the softmax denominator)
      y_h^T = out^T[:dk] * bcast(1/denominator)   (deferred off critical path)
  - out_partial[T, D] = Y^T.T @ Wo_block^T  (deferred/interleaved; host adds
    the 4 partials + bo)
"""

import os
import numpy as np

import concourse.bacc as bacc
import concourse.mybir as mybir
import concourse.tile as tile
from concourse.bass_utils import run_bass_kernel_spmd

F32 = mybir.dt.float32
F32R = mybir.dt.float32r

B, T, D, H = 2, 2048, 1024, 16
DK = 64                 # head dim
NH_LOC = 4              # heads per core
HD = NH_LOC * DK        # 256 local head dims
N_CORES = 8
QB = 512                # q block width
N_QB = T // QB          # 4
N_KT = T // 128         # 16 kv tiles

# matmul input mode: "f32r" (fast, reduced-precision PE input) or "f32" (4x slower)
MM_MODE = os.environ.get("ATT_MM_MODE", "f32r")
MD = F32R if MM_MODE == "f32r" else F32


def _build():
    nc = bacc.Bacc()

    xT = nc.dram_tensor("xT", [D, T], MD, kind="ExternalInput")
    wqT = nc.dram_tensor("wqT", [D, HD], MD, kind="ExternalInput")
    wkT = nc.dram_tensor("wkT", [D, HD], MD, kind="ExternalInput")
    wvT = nc.dram_tensor("wvT", [D, HD], MD, kind="ExternalInput")
    woT = nc.dram_tensor("woT", [HD, D], MD, kind="ExternalInput")
    bq = nc.dram_tensor("bq", [HD], F32, kind="ExternalInput")
    bk = nc.dram_tensor("bk", [HD], F32, kind="ExternalInput")
    bv_bc = nc.dram_tensor("bv_bc", [128, HD], MD, kind="ExternalInput")
    ones_row = nc.dram_tensor("ones_row", [1, 128], MD, kind="ExternalInput")
    ones_col = nc.dram_tensor("ones_col", [128, DK], MD, kind="ExternalInput")
    out = nc.dram_tensor("out", [T, D], F32, kind="ExternalOutput")

    ident = mybir.ActivationFunctionType.Identity
    exp = mybir.ActivationFunctionType.Exp
    recip_mode = os.environ.get("ATT_RECIP", "approx")

    with nc.allow_low_precision(reason="fp32r matmul inputs; fp32 accumulate"), \
         tile.TileContext(nc) as tc:
        with tc.tile_pool(name="persist", bufs=1) as persist, \
             tc.tile_pool(name="xc_pool", bufs=2) as xcp, \
             tc.tile_pool(name="att_sb", bufs=4) as asb, \
             tc.tile_pool(name="att_ps", bufs=3, space="PSUM") as aps, \
             tc.tile_pool(name="acc_ps", bufs=2, space="PSUM") as ops:
            # ---- persistent SBUF tensors ----
            wqT_sb = persist.tile([128, 8, HD], MD, tag="wq")      # 8KB/par
            wkT_sb = persist.tile([128, 8, HD], MD, tag="wk")
            wvT_sb = persist.tile([128, 8, HD], MD, tag="wv")
            woT_sb = persist.tile([128, 2, D], MD, tag="wo")       # 8KB/par
            bq_sb = persist.tile([128, 2], F32, tag="bq")
            bk_sb = persist.tile([128, 2], F32, tag="bk")
            bvb_sb = persist.tile([128, HD], MD, tag="bvb")
            ones_sb = persist.tile([1, 128], MD, tag="ones")
            QT_sb = persist.tile([128, 2, T], MD, tag="QT")        # 16KB/par
            KT_sb = persist.tile([128, 2, T], MD, tag="KT")
            YT_sb = persist.tile([128, 2, T], MD, tag="YT")
            V_sb = persist.tile([128, N_KT, NH_LOC, DK + 1], MD, tag="V")

            # ---- parameter loads (x^T chunks stream inside the main loop) ----
            xT_re = xT[:].rearrange("(t p) n -> p t n", p=128)
            xcs = {}
            xcs[0] = xcp.tile([128, 8, QB], MD, tag="xc", name="xc")
            wq_re = wqT[:].rearrange("(t p) n -> p t n", p=128)
            for k0 in range(0, 8, 2):
                nc.sync.dma_start(out=xcs[0][:, k0:k0 + 2, :], in_=xT_re[:, k0:k0 + 2, 0:QB])
                nc.sync.dma_start(out=wqT_sb[:, k0:k0 + 2, :], in_=wq_re[:, k0:k0 + 2, :])
            nc.sync.dma_start(out=bq_sb[:], in_=bq[:].rearrange("(m p) -> p m", p=128))
            nc.sync.dma_start(out=wkT_sb[:], in_=wkT[:].rearrange("(t p) n -> p t n", p=128))
            nc.sync.dma_start(out=bk_sb[:], in_=bk[:].rearrange("(m p) -> p m", p=128))
            nc.sync.dma_start(out=wvT_sb[:], in_=wvT[:].rearrange("(t p) n -> p t n", p=128))
            nc.sync.dma_start(out=bvb_sb[:], in_=bv_bc[:])
            nc.sync.dma_start(out=ones_sb[:], in_=ones_row[:])
            nc.sync.dma_start(out=V_sb[:, :, :, DK], in_=ones_col[:])
            nc.sync.dma_start(out=woT_sb[:], in_=woT[:].rearrange("(t p) n -> p t n", p=128))

            pending = []

            def _proj(n):
                # q/k/v projections for T columns [n*QB, (n+1)*QB)
                xc = xcs.pop(n)
                if n + 1 < N_QB:
                    xcs[n + 1] = xcp.tile([128, 8, QB], MD, tag="xc", name="xc")
                    nc.sync.dma_start(out=xcs[n + 1][:],
                                      in_=xT_re[:, :, (n + 1) * QB:(n + 2) * QB])
                for wT_sb, b_sb, dst in ((wqT_sb, bq_sb, QT_sb), (wkT_sb, bk_sb, KT_sb)):
                    for m in range(2):
                        ps = aps.tile([128, QB], F32, tag="sps", name="psqk")
                        for k8 in range(8):
                            nc.tensor.matmul(
                                ps[:],
                                wT_sb[:, k8, m * 128:(m + 1) * 128],
                                xc[:, k8, :],
                                start=(k8 == 0), stop=(k8 == 7),
                            )
                        nc.scalar.activation(
                            dst[:, m, n * QB:(n + 1) * QB], ps[:],
                            ident, bias=b_sb[:, m:m + 1],
                        )
                for tt in range(4):
                    t = 4 * n + tt
                    ps = aps.tile([128, HD], F32, tag="sps", name="psv")
                    for k8 in range(8):
                        nc.tensor.matmul(
                            ps[:],
                            xc[:, k8, tt * 128:(tt + 1) * 128],
                            wvT_sb[:, k8, :],
                            start=(k8 == 0), stop=(k8 == 7),
                        )
                    nc.vector.tensor_tensor(
                        V_sb[:, t, :, 0:DK],
                        ps[:].rearrange("p (h d) -> p h d", h=NH_LOC),
                        bvb_sb[:].rearrange("p (h d) -> p h d", h=NH_LOC),
                        op=mybir.AluOpType.add,
                    )

            def _norm(h, ti, qb, outp, late=False):
                base = (h % 2) * 64
                yslice = YT_sb[base:base + 64, ti, qb * QB:(qb + 1) * QB]
                cp = nc.scalar.copy if late else nc.vector.tensor_copy
                cp(yslice, outp[0:DK, :])
                sums_f = asb.tile([1, QB], F32, tag="sums", name="sums")
                cp(sums_f[:], outp[DK:DK + 1, :])
                recip = asb.tile([1, QB], MD, tag="recip", name="recip")
                if recip_mode == "approx":
                    recip_f = asb.tile([1, QB], F32, tag="recipf", name="recipf")
                    nc.vector.reciprocal_approx_fast(recip_f[:], sums_f[:])
                    nc.vector.tensor_copy(recip[:], recip_f[:])
                else:
                    nc.vector.reciprocal(recip[:], sums_f[:])
                bc_sb = asb.tile([128, QB], MD, tag="bcs", name="bcs")
                nc.gpsimd.partition_broadcast(bc_sb[:], recip[:])
                nc.vector.tensor_mul(yslice, yslice, bc_sb[base:base + 64, :])

            def _outproj(qb):
                for t in range(4 * qb, 4 * qb + 4):
                    res = asb.tile([128, D], F32, tag="res", name="res")
                    for n2 in range(2):
                        ps = aps.tile([128, QB], F32, tag="sps", name="pso")
                        for k2 in range(2):
                            nc.tensor.matmul(
                                ps[:],
                                YT_sb[:, k2, t * 128:(t + 1) * 128],
                                woT_sb[:, k2, n2 * QB:(n2 + 1) * QB],
                                start=(k2 == 0), stop=(k2 == 1),
                            )
                        nc.vector.tensor_copy(res[:, n2 * QB:(n2 + 1) * QB], ps[:])
                    nc.sync.dma_start(out=out[t * 128:(t + 1) * 128, :], in_=res[:])

            # ---- main loop: projections for chunk qb, then attention on it ----
            for qb in range(N_QB):
                _proj(qb)
                for ti in range(2):
                    heads = (2 * ti, 2 * ti + 1)
                    outps = {}
                    for h in heads:
                        outps[h] = ops.tile([DK + 1, QB], F32, tag="outp", name="outp")
                    n_kv = 4 * (qb + 1)
                    npairs = n_kv // 2
                    sps = {}

                    def _emit_qk(pj, ti=ti, qb=qb, heads=heads):
                        kt0 = 2 * pj
                        q0 = max(kt0 * 128 - qb * QB, 0)
                        for h in heads:
                            sps[(h, pj)] = aps.tile([128, 2, QB], F32,
                                                    tag="sps", name="sps")
                        for j in range(2):
                            for h in heads:
                                base = (h % 2) * 64
                                nc.tensor.matmul(
                                    sps[(h, pj)][:, j, q0:],
                                    KT_sb[base:base + 64, ti, (kt0 + j) * 128:(kt0 + j + 1) * 128],
                                    QT_sb[base:base + 64, ti, qb * QB + q0:(qb + 1) * QB],
                                    start=True, stop=True,
                                )

                    _emit_qk(0)
                    for pj in range(npairs):
                        if pj + 1 < npairs:
                            _emit_qk(pj + 1)
                        if pj == min(1, npairs - 1):
                            for fn in pending:
                                fn()
                            pending = []
                        kt0 = 2 * pj
                        q0 = max(kt0 * 128 - qb * QB, 0)
                        for h in heads:
                            sp = sps.pop((h, pj))
                            pT = asb.tile([128, 2, QB], MD, tag="pT")
                            nc.scalar.activation(pT[:, :, q0:], sp[:, :, q0:],
                                                 exp, scale=0.125)
                            for j in range(2):
                                r = (kt0 + j) * 128 - qb * QB
                                if r >= 0:
                                    nc.gpsimd.affine_select(
                                        out=pT[:, j, q0:], in_=pT[:, j, q0:],
                                        compare_op=mybir.AluOpType.is_ge,
                                        fill=0.0, base=q0 - r, channel_multiplier=-1,
                                        pattern=[[1, QB - q0]],
                                    )
                            for j in range(2):
                                nc.tensor.matmul(
                                    outps[h][:, q0:],
                                    V_sb[:, kt0 + j, h, :],
                                    pT[:, j, q0:],
                                    start=(kt0 + j == 0), stop=(kt0 + j == n_kv - 1),
                                )
                    late = (qb == N_QB - 1 and ti == 1)
                    for h in heads:
                        pending.append(
                            lambda h=h, ti=ti, qb=qb, outp=outps[h], late=late:
                                _norm(h, ti, qb, outp, late))
                    if ti == 1:
                        pending.append(lambda qb=qb: _outproj(qb))
            for fn in pending:
                fn()

    nc.finalize()
    return nc


_NC = None


def _get_nc():
    global _NC
    if _NC is None:
        _NC = _build()
    return _NC


def _shard_inputs(x, wq, bq, wk, bk, wv, bv, wo):
    in_maps = []
    for c in range(N_CORES):
        b = c // 4
        sl = slice((c % 4) * HD, (c % 4 + 1) * HD)
        in_maps.append({
            "xT": np.ascontiguousarray(x[b].T),
            "wqT": np.ascontiguousarray(wq[sl].T),
            "wkT": np.ascontiguousarray(wk[sl].T),
            "wvT": np.ascontiguousarray(wv[sl].T),
            "woT": np.ascontiguousarray(wo[:, sl].T),
            "bq": np.ascontiguousarray(bq[sl]),
            "bk": np.ascontiguousarray(bk[sl]),
            "bv_bc": np.ascontiguousarray(np.broadcast_to(bv[sl][None, :], (128, HD))),
            "ones_row": np.ones((1, 128), dtype=np.float32),
            "ones_col": np.ones((128, DK), dtype=np.float32),
        })
    return in_maps


def kernel(x, wq, bq, wk, bk, wv, bv, wo, bo, _trace=False, **_trace_kw):
    x = np.asarray(x, dtype=np.float32)
    nc = _get_nc()
    in_maps = _shard_inputs(
        x, np.asarray(wq), np.asarray(bq), np.asarray(wk), np.asarray(bk),
        np.asarray(wv), np.asarray(bv), np.asarray(wo))
    res = run_bass_kernel_spmd(nc, in_maps, list(range(N_CORES)),
                               trace=_trace, **_trace_kw)
    parts = [res.results[c]["out"] for c in range(N_CORES)]
    bo = np.asarray(bo, dtype=np.float32)
    y = np.stack([
        parts[0] + parts[1] + parts[2] + parts[3] + bo,
        parts[4] + parts[5] + parts[6] + parts[7] + bo,
    ]).astype(np.float32)
    if _trace:
        kernel.last_results = res
    return y
